# revision 24
# baseline (speedup 1.0000x reference)
"""Masked dot-product attention on 8 Trainium2 NeuronCores.

Problem: q,k,v [64, 1024, 64] f32, valid_lens [64] int32.
  scores = q @ k^T / 8, mask keys >= valid_len to -1e6, softmax, @ v.

Strategy (per core: 8 batches, pure data parallelism, no collectives):
  - Host prep: pre-transpose q,k to [D, S] (contraction dim on partitions),
    pre-zero v rows past valid_len and append the 0/1 mask as a 65th column
    (vm).  The masked softmax denominator then falls out of the same matmul
    that computes attn @ v.  valid_len==0 batches reproduce the reference's
    uniform-softmax by zeroing q (scores==0) and unmasking all keys.
  - Matmul dtypes, chosen off measured TRN2 PE rates: all matmuls run
    f32r x f32r (fp32 bit layout at 1.5 cycles/row vs fp32's 4; bf16 would
    be 1 cycle/row but its 8-bit mantissa costs ~2e-3 output error), so
    q/k/v/exp are never quantized below fp32.
  - Device, per key-tile j: scoresT[j,q] = kT_tile.T.T @ qT (keys on
    partitions, fp32 PSUM), exp on ScalarE (scale=1/8, bias=-3 bounds the
    fp16 range; numerator and denominator share it so it cancels), then
    po[65, Sq] += vm_tile.T.T @ expT accumulated over j in fp32 PSUM.
    No max-subtraction: scores are O(1) (q,k ~ N(0,1), d=64), and masked
    columns are excluded by the mask column/zeroed v rows, not by -1e6.
  - Transpose-free epilogue: reciprocal_approx_fast over the denominator
    row (PSUM -> SBUF), GpSimd partition_broadcast across 64 partitions,
    one tensor_tensor multiply normalizes the [64, Sq] block, DMA out in
    [d, q] layout; the host transposes each batch during the gather.
  - DMA dispatch is the hidden serial resource (~0.6us per dma_start on a
    sequencer): inputs ride the Sync queue, epilogue DMAs the GpSimd queue.
  - Per-batch key tiles are truncated to ceil(valid/128): masked tail tiles
    contribute exactly zero, so they are skipped.  Batches are rank-sorted by
    valid_len and dealt one per core per slot so every core runs the same
    baked schedule with minimal padding.
"""

import numpy as np

import concourse.bass as bass
import concourse.bacc as bacc
import concourse.tile as tile
from concourse import mybir
from concourse import bass_utils

B, S, D = 64, 1024, 64
NCORES = 8
NB = B // NCORES  # batch slots per core
P = 128
NJT = S // P  # max key tiles per batch
W = D + 1  # v columns + mask column
F32 = mybir.dt.float32
F32R = mybir.dt.float32r
F16 = mybir.dt.float16

TRACE = False  # set by test harness to capture an NTFF profile
LAST_RESULTS = None  # BassKernelResults stash for the harness

_program_cache = {}


def _build_program(jt_counts):
    nc = bacc.Bacc("TRN2", target_bir_lowering=False, debug=False,
                   num_devices=NCORES)
    qT = nc.dram_tensor("qT", [NB, D, S], F32R, kind="ExternalInput").ap()
    kT = nc.dram_tensor("kT", [NB, D, S], F32R, kind="ExternalInput").ap()
    vm = nc.dram_tensor("vm", [NB, S, W], F32R, kind="ExternalInput").ap()
    out = nc.dram_tensor("outT", [NB, D, S], F32, kind="ExternalOutput").ap()

    with tile.TileContext(nc) as tc:
        with (
            tc.tile_pool(name="singles", bufs=1) as singles,
            tc.tile_pool(name="qk", bufs=2) as qk_pool,
            tc.tile_pool(name="vmp", bufs=2) as vm_pool,
            tc.tile_pool(name="ex", bufs=4) as ex_pool,
            tc.tile_pool(name="rcp", bufs=2) as rcp_pool,
            tc.tile_pool(name="scr", bufs=2, space="DRAM") as scr_pool,
            tc.tile_pool(name="rbc", bufs=2) as rbc_pool,
            tc.tile_pool(name="osb", bufs=2) as osb_pool,
            tc.tile_pool(name="ps_s", bufs=2, space="PSUM") as ps_pool,
            tc.tile_pool(name="ps_o", bufs=2, space="PSUM") as po_pool,
        ):
            # exp(s/8 - 3): the -3 bounds the fp16 exp range; it cancels
            # between numerator and denominator.
            bias_t = singles.tile([P, 1], F32)
            nc.vector.memset(bias_t, -3.0)

            for s in range(NB):
                jt = jt_counts[s]
                qT_t = qk_pool.tile([D, S], F32R, tag="qT")
                kT_t = qk_pool.tile([D, S], F32R, tag="kT")
                nc.sync.dma_start(out=qT_t, in_=qT[s])
                nc.sync.dma_start(out=kT_t[:, 0:jt * P], in_=kT[s, :, 0:jt * P])
                # All key tiles of vm in one DMA: [128, jt*65], tile j at
                # columns [j*65, (j+1)*65).
                vm_t = vm_pool.tile([P, NJT * W], F32R, tag="vm", name="vm_t")
                nc.sync.dma_start(
                    out=vm_t.rearrange("p (j w) -> p j w", w=W)[:, 0:jt, :],
                    in_=vm[s, 0:jt * P, :].rearrange("(j p) w -> p j w", p=P),
                )
                # [v|mask]^T-weighted sums: rows 0..63 unnormalized outT,
                # row 64 the softmax denominator.  One accumulation group
                # per PSUM bank (cols 0:512 and 512:1024), spanning all j.
                po = po_pool.tile([W, S], F32, name="po")

                def emit_av(ex_j, j, jt=jt, po=po, vm_t=vm_t):
                    for half in range(2):
                        nc.tensor.matmul(
                            po[:, half * 512:(half + 1) * 512],
                            lhsT=vm_t[:, j * W:(j + 1) * W],
                            rhs=ex_j[:, half * 512:(half + 1) * 512],
                            start=(j == 0), stop=(j == jt - 1),
                        )

                # Scores/exp run one j ahead of the attn@v accumulation so
                # the PE never sits behind ScalarE in its own queue.
                prev = None
                for j in range(jt):
                    ps = ps_pool.tile([P, S], F32, tag="ps")
                    for half in range(2):
                        nc.tensor.matmul(
                            ps[:, half * 512:(half + 1) * 512],
                            lhsT=kT_t[:, j * P:(j + 1) * P],
                            rhs=qT_t[:, half * 512:(half + 1) * 512],
                            start=True, stop=True,
                        )
                    ex = ex_pool.tile([P, S], F32R, tag="ex", name="ex")
                    nc.scalar.activation(out=ex, in_=ps,
                                         func=mybir.ActivationFunctionType.Exp,
                                         scale=0.125, bias=bias_t)
                    if prev is not None:
                        emit_av(*prev)
                    prev = (ex, j)
                emit_av(*prev)

                # 1/denominator (18-bit approx is ~5x faster than the
                # iterative divide and den is a tame [1.1, 127] range), then
                # replicate across the 64 output partitions on GpSimd.
                # DVE reciprocal runs 8 cycles per element serially along
                # the free dim, so a [1, 1024] denominator row would cost
                # ~6.5us.  Bounce it through DRAM to fold it onto 128
                # partitions ([128, 8] costs ~0.2us), invert, bounce back to
                # a row, and broadcast across the 64 output partitions
                # (0-stride reads are legal on DRAM sources).
                dsb = rcp_pool.tile([1, S], F32, tag="dsb", name="dsb")
                nc.vector.tensor_copy(out=dsb, in_=po[D:W, :])
                scr = scr_pool.tile([1, S], F32, tag="scr", name="scr")
                nc.gpsimd.dma_start(out=scr, in_=dsb)
                d128 = rcp_pool.tile([P, S // P], F32, tag="d128", name="d128")
                scr_pc = bass.AP(tensor=scr.tensor, offset=scr.offset,
                                 ap=[[1, P], [P, S // P]])
                nc.gpsimd.dma_start(out=d128, in_=scr_pc)
                r128 = rcp_pool.tile([P, S // P], F32, tag="r128", name="r128")
                nc.vector.reciprocal(out=r128, in_=d128)
                scr2 = scr_pool.tile([1, S], F32, tag="scr2", name="scr2")
                scr2_pc = bass.AP(tensor=scr2.tensor, offset=scr2.offset,
                                  ap=[[1, P], [P, S // P]])
                nc.gpsimd.dma_start(out=scr2_pc, in_=r128)
                rbc = rbc_pool.tile([D, S], F32, tag="rbc", name="rbc")
                bcast_src = bass.AP(tensor=scr2.tensor, offset=scr2.offset,
                                    ap=[[0, D], [1, S]])
                nc.gpsimd.dma_start(out=rbc, in_=bcast_src)
                osb = osb_pool.tile([D, S], F32, tag="osb", name="osb")
                nc.vector.tensor_mul(osb, po[0:D, :], rbc)
                nc.gpsimd.dma_start(out=out[s], in_=osb)
    nc.compile()
    return nc


def kernel(q, k, v, valid_lens):
    global LAST_RESULTS
    q = np.array(q, dtype=np.float32, copy=True)
    k = np.asarray(k, dtype=np.float32)
    v = np.asarray(v, dtype=np.float32)
    vl = np.asarray(valid_lens).astype(np.int64)

    # valid_len == 0: reference's softmax over an all-masked row is uniform.
    # Zeroed q gives scores == 0 -> exp == 1 over all (unmasked) keys: same.
    valid_eff = np.where(vl <= 0, S, np.minimum(vl, S))
    q[vl <= 0] = 0.0

    mask = (np.arange(S)[None, :] < valid_eff[:, None]).astype(np.float32)
    qT = np.ascontiguousarray(q.transpose(0, 2, 1))
    kT = np.ascontiguousarray(k.transpose(0, 2, 1))
    vm = np.concatenate([v * mask[:, :, None], mask[:, :, None]], axis=2)
    vm = np.ascontiguousarray(vm, dtype=np.float32)

    # Rank-sort batches by effective length; slot s takes ranks [8s, 8s+8),
    # one per core, so the baked per-slot tile count wastes little work.
    order = np.argsort(-valid_eff, kind="stable")
    assign = order.reshape(NB, NCORES)  # [slot, core] -> batch index
    jt_counts = tuple(
        int(np.ceil(valid_eff[assign[s]].max() / P)) for s in range(NB)
    )

    nc = _program_cache.get(jt_counts)
    if nc is None:
        nc = _build_program(jt_counts)
        _program_cache[jt_counts] = nc

    in_maps = []
    for c in range(NCORES):
        bs = assign[:, c]
        in_maps.append({
            "qT": np.ascontiguousarray(qT[bs]),
            "kT": np.ascontiguousarray(kT[bs]),
            "vm": np.ascontiguousarray(vm[bs]),
        })
    res = bass_utils.run_bass_kernel_spmd(
        nc, in_maps, core_ids=list(range(NCORES)), trace=TRACE,
    )
    LAST_RESULTS = res

    out = np.empty((B, S, D), dtype=np.float32)
    for c in range(NCORES):
        o = res.results[c]["outT"]  # [NB, D, S]
        for s in range(NB):
            out[assign[s, c]] = o[s].T
    return out


# revision 26
# speedup vs baseline: 1.0312x; 1.0312x over previous
"""Masked dot-product attention on 8 Trainium2 NeuronCores.

Problem: q,k,v [64, 1024, 64] f32, valid_lens [64] int32.
  scores = q @ k^T / 8, mask keys >= valid_len to -1e6, softmax, @ v.

Strategy (per core: 8 batches, pure data parallelism, no collectives):
  - Host prep: pre-transpose q,k to [D, S] (contraction dim on partitions),
    pre-zero v rows past valid_len and append the 0/1 mask as a 65th column
    (vm).  The masked softmax denominator then falls out of the same matmul
    that computes attn @ v.  valid_len==0 batches reproduce the reference's
    uniform-softmax by zeroing q (scores==0) and unmasking all keys.
  - Matmul dtypes, chosen off measured TRN2 PE rates: all matmuls run
    f32r x f32r (fp32 bit layout at 1.5 cycles/row vs fp32's 4; bf16 would
    be 1 cycle/row but its 8-bit mantissa costs ~2e-3 output error), so
    q/k/v/exp are never quantized below fp32.
  - Device, per key-tile j: scoresT[j,q] = kT_tile.T.T @ qT (keys on
    partitions, fp32 PSUM), exp on ScalarE (scale=1/8, bias=-3 bounds the
    fp16 range; numerator and denominator share it so it cancels), then
    po[65, Sq] += vm_tile.T.T @ expT accumulated over j in fp32 PSUM.
    No max-subtraction: scores are O(1) (q,k ~ N(0,1), d=64), and masked
    columns are excluded by the mask column/zeroed v rows, not by -1e6.
  - Transpose-free epilogue: reciprocal_approx_fast over the denominator
    row (PSUM -> SBUF), GpSimd partition_broadcast across 64 partitions,
    one tensor_tensor multiply normalizes the [64, Sq] block, DMA out in
    [d, q] layout; the host transposes each batch during the gather.
  - DMA dispatch is the hidden serial resource (~0.6us per dma_start on a
    sequencer): inputs ride the Sync queue, epilogue DMAs the GpSimd queue.
  - Per-batch key tiles are truncated to ceil(valid/128): masked tail tiles
    contribute exactly zero, so they are skipped.  Batches are rank-sorted by
    valid_len and dealt one per core per slot so every core runs the same
    baked schedule with minimal padding.
"""

import numpy as np

import concourse.bass as bass
import concourse.bacc as bacc
import concourse.tile as tile
from concourse import mybir
from concourse import bass_utils

B, S, D = 64, 1024, 64
NCORES = 8
NB = B // NCORES  # batch slots per core
P = 128
NJT = S // P  # max key tiles per batch
W = D + 1  # v columns + mask column
F32 = mybir.dt.float32
F32R = mybir.dt.float32r
F16 = mybir.dt.float16

TRACE = False  # set by test harness to capture an NTFF profile
LAST_RESULTS = None  # BassKernelResults stash for the harness

_program_cache = {}


def _build_program(jt_counts):
    nc = bacc.Bacc("TRN2", target_bir_lowering=False, debug=False,
                   num_devices=NCORES)
    qT = nc.dram_tensor("qT", [NB, D, S], F32R, kind="ExternalInput").ap()
    kT = nc.dram_tensor("kT", [NB, D, S], F32R, kind="ExternalInput").ap()
    vm = nc.dram_tensor("vm", [NB, S, W], F32R, kind="ExternalInput").ap()
    out = nc.dram_tensor("outT", [NB, D, S], F32, kind="ExternalOutput").ap()

    with tile.TileContext(nc) as tc:
        with (
            tc.tile_pool(name="singles", bufs=1) as singles,
            tc.tile_pool(name="qk", bufs=2) as qk_pool,
            tc.tile_pool(name="vmp", bufs=2) as vm_pool,
            tc.tile_pool(name="ex", bufs=4) as ex_pool,
            tc.tile_pool(name="so", bufs=2) as so_pool,
            tc.tile_pool(name="rcp", bufs=2) as rcp_pool,
            tc.tile_pool(name="scr", bufs=2, space="DRAM") as scr_pool,
            tc.tile_pool(name="rbc", bufs=2) as rbc_pool,
            tc.tile_pool(name="osb", bufs=2) as osb_pool,
            tc.tile_pool(name="ps_s", bufs=2, space="PSUM") as ps_pool,
            tc.tile_pool(name="ps_o", bufs=1, space="PSUM") as po_pool,
            tc.tile_pool(name="ps_t", bufs=2, space="PSUM") as pt_pool,
        ):
            # exp(s/8 - 3): the -3 bounds the denominator's fp16 range; it
            # cancels between numerator and denominator.
            bias_t = singles.tile([P, 1], F32)
            nc.vector.memset(bias_t, -3.0)
            ident_t = singles.tile([1, 1], F16)
            nc.vector.memset(ident_t, 1.0)

            for s in range(NB):
                jt = jt_counts[s]
                qT_t = qk_pool.tile([D, S], F32R, tag="qT")
                kT_t = qk_pool.tile([D, S], F32R, tag="kT")
                nc.sync.dma_start(out=qT_t, in_=qT[s])
                nc.sync.dma_start(out=kT_t[:, 0:jt * P], in_=kT[s, :, 0:jt * P])
                # All key tiles of vm in one DMA: [128, jt*65], tile j at
                # columns [j*65, (j+1)*65).
                vm_t = vm_pool.tile([P, NJT * W], F32R, tag="vm", name="vm_t")
                nc.sync.dma_start(
                    out=vm_t.rearrange("p (j w) -> p j w", w=W)[:, 0:jt, :],
                    in_=vm[s, 0:jt * P, :].rearrange("(j p) w -> p j w", p=P),
                )
                # [v|mask]^T-weighted sums: rows 0..63 unnormalized outT,
                # row 64 the softmax denominator.  One accumulation group
                # per PSUM bank (cols 0:512 and 512:1024), spanning all j.
                po = po_pool.tile([W, S], F32, name="po")

                def emit_av(ex_j, j, jt=jt, po=po, vm_t=vm_t):
                    for half in range(2):
                        nc.tensor.matmul(
                            po[:, half * 512:(half + 1) * 512],
                            lhsT=vm_t[:, j * W:(j + 1) * W],
                            rhs=ex_j[:, half * 512:(half + 1) * 512],
                            start=(j == 0), stop=(j == jt - 1),
                        )

                # Scores/exp run one j ahead of the attn@v accumulation so
                # the PE never sits behind ScalarE in its own queue.
                prev = None
                for j in range(jt):
                    ps = ps_pool.tile([P, S], F32, tag="ps")
                    for half in range(2):
                        nc.tensor.matmul(
                            ps[:, half * 512:(half + 1) * 512],
                            lhsT=kT_t[:, j * P:(j + 1) * P],
                            rhs=qT_t[:, half * 512:(half + 1) * 512],
                            start=True, stop=True,
                        )
                    ex = ex_pool.tile([P, S], F32R, tag="ex", name="ex")
                    nc.scalar.activation(out=ex, in_=ps,
                                         func=mybir.ActivationFunctionType.Exp,
                                         scale=0.125, bias=bias_t)
                    if prev is not None:
                        emit_av(*prev)
                    prev = (ex, j)
                emit_av(*prev)

                # 1/denominator (18-bit approx is ~5x faster than the
                # iterative divide and den is a tame [1.1, 127] range), then
                # replicate across the 64 output partitions on GpSimd.
                # Free po as fast as possible: one DVE copy moves the whole
                # accumulator to SBUF, so the next batch's matmuls only wait
                # ~1.2us instead of on the full normalization chain.
                ub = so_pool.tile([W, S], F32, tag="ub", name="ub")
                nc.vector.tensor_copy(out=ub, in_=po)
                # DVE reciprocal runs 8 cycles/element serially along the
                # free dim ([1, 1024] would be ~6.5us), so fold the
                # denominator row onto 128 partitions first: copy it to fp16
                # (cheap PE weight loads), transpose 128-wide slices into one
                # PSUM bank, invert as [128, 8] (~0.2us), then bounce through
                # DRAM to restore row layout broadcast over 64 partitions
                # (0-stride reads are legal on DRAM sources).
                dsb = rcp_pool.tile([1, S], F16, tag="dsb", name="dsb")
                nc.vector.tensor_copy(out=dsb, in_=po[D:W, :])
                pt = pt_pool.tile([P, 2 * (S // P)], F16, tag="pt", name="pt")
                for c in range(S // P):
                    # column 2c keeps each fp16 write 4-byte aligned
                    nc.tensor.transpose(pt[:, 2 * c:2 * c + 1],
                                        dsb[:, c * P:(c + 1) * P], ident_t)
                r128 = rcp_pool.tile([P, S // P], F32, tag="r128", name="r128")
                nc.vector.reciprocal(
                    out=r128,
                    in_=pt.rearrange("p (c two) -> p c two", two=2)[:, :, 0])
                scr2 = scr_pool.tile([1, S], F32, tag="scr2", name="scr2")
                scr2_pc = bass.AP(tensor=scr2.tensor, offset=scr2.offset,
                                  ap=[[1, P], [P, S // P]])
                nc.gpsimd.dma_start(out=scr2_pc, in_=r128)
                rbc = rbc_pool.tile([D, S], F32, tag="rbc", name="rbc")
                bcast_src = bass.AP(tensor=scr2.tensor, offset=scr2.offset,
                                    ap=[[0, D], [1, S]])
                nc.gpsimd.dma_start(out=rbc, in_=bcast_src)
                osb = osb_pool.tile([D, S], F32, tag="osb", name="osb")
                nc.vector.tensor_mul(osb, ub[0:D, :], rbc)
                nc.sync.dma_start(out=out[s], in_=osb)
    nc.compile()
    return nc


def kernel(q, k, v, valid_lens):
    global LAST_RESULTS
    q = np.array(q, dtype=np.float32, copy=True)
    k = np.asarray(k, dtype=np.float32)
    v = np.asarray(v, dtype=np.float32)
    vl = np.asarray(valid_lens).astype(np.int64)

    # valid_len == 0: reference's softmax over an all-masked row is uniform.
    # Zeroed q gives scores == 0 -> exp == 1 over all (unmasked) keys: same.
    valid_eff = np.where(vl <= 0, S, np.minimum(vl, S))
    q[vl <= 0] = 0.0

    mask = (np.arange(S)[None, :] < valid_eff[:, None]).astype(np.float32)
    qT = np.ascontiguousarray(q.transpose(0, 2, 1))
    kT = np.ascontiguousarray(k.transpose(0, 2, 1))
    vm = np.concatenate([v * mask[:, :, None], mask[:, :, None]], axis=2)
    vm = np.ascontiguousarray(vm, dtype=np.float32)

    # Rank-sort batches by effective length; slot s takes ranks [8s, 8s+8),
    # one per core, so the baked per-slot tile count wastes little work.
    order = np.argsort(-valid_eff, kind="stable")
    assign = order.reshape(NB, NCORES)  # [slot, core] -> batch index
    jt_counts = tuple(
        int(np.ceil(valid_eff[assign[s]].max() / P)) for s in range(NB)
    )

    nc = _program_cache.get(jt_counts)
    if nc is None:
        nc = _build_program(jt_counts)
        _program_cache[jt_counts] = nc

    in_maps = []
    for c in range(NCORES):
        bs = assign[:, c]
        in_maps.append({
            "qT": np.ascontiguousarray(qT[bs]),
            "kT": np.ascontiguousarray(kT[bs]),
            "vm": np.ascontiguousarray(vm[bs]),
        })
    res = bass_utils.run_bass_kernel_spmd(
        nc, in_maps, core_ids=list(range(NCORES)), trace=TRACE,
    )
    LAST_RESULTS = res

    out = np.empty((B, S, D), dtype=np.float32)
    for c in range(NCORES):
        o = res.results[c]["outT"]  # [NB, D, S]
        for s in range(NB):
            out[assign[s, c]] = o[s].T
    return out


# revision 28
# speedup vs baseline: 1.1578x; 1.1228x over previous
"""Masked dot-product attention on 8 Trainium2 NeuronCores.

Problem: q,k,v [64, 1024, 64] f32, valid_lens [64] int32.
  scores = q @ k^T / 8, mask keys >= valid_len to -1e6, softmax, @ v.

Strategy (per core: 8 batches, pure data parallelism, no collectives):
  - Host prep: pre-transpose q,k to [D, S] (contraction dim on partitions),
    pre-zero v rows past valid_len and append the 0/1 mask as a 65th column
    (vm).  The masked softmax denominator then falls out of the same matmul
    that computes attn @ v.  valid_len==0 batches reproduce the reference's
    uniform-softmax by zeroing q (scores==0) and unmasking all keys.
  - Matmul dtypes, chosen off measured TRN2 PE rates: all matmuls run
    f32r x f32r (fp32 bit layout at 1.5 cycles/row vs fp32's 4; bf16 would
    be 1 cycle/row but its 8-bit mantissa costs ~2e-3 output error), so
    q/k/v/exp are never quantized below fp32.
  - Device, per key-tile j: scoresT[j,q] = kT_tile.T.T @ qT (keys on
    partitions, fp32 PSUM), exp on ScalarE (scale=1/8, bias=-3 bounds the
    fp16 range; numerator and denominator share it so it cancels), then
    po[65, Sq] += vm_tile.T.T @ expT accumulated over j in fp32 PSUM.
    No max-subtraction: scores are O(1) (q,k ~ N(0,1), d=64), and masked
    columns are excluded by the mask column/zeroed v rows, not by -1e6.
  - Transpose-free epilogue: reciprocal_approx_fast over the denominator
    row (PSUM -> SBUF), GpSimd partition_broadcast across 64 partitions,
    one tensor_tensor multiply normalizes the [64, Sq] block, DMA out in
    [d, q] layout; the host transposes each batch during the gather.
  - DMA dispatch is the hidden serial resource (~0.6us per dma_start on a
    sequencer): inputs ride the Sync queue, epilogue DMAs the GpSimd queue.
  - Per-batch key tiles are truncated to ceil(valid/128): masked tail tiles
    contribute exactly zero, so they are skipped.  Batches are rank-sorted by
    valid_len and dealt one per core per slot so every core runs the same
    baked schedule with minimal padding.
"""

import numpy as np

import concourse.bass as bass
import concourse.bacc as bacc
import concourse.tile as tile
from concourse import mybir
from concourse import bass_utils

B, S, D = 64, 1024, 64
NCORES = 8
NB = B // NCORES  # batch slots per core
P = 128
NJT = S // P  # max key tiles per batch
W = D + 1  # v columns + mask column
F32 = mybir.dt.float32
F32R = mybir.dt.float32r
F16 = mybir.dt.float16

TRACE = False  # set by test harness to capture an NTFF profile
LAST_RESULTS = None  # BassKernelResults stash for the harness

_program_cache = {}


def _build_program(jt_counts):
    nc = bacc.Bacc("TRN2", target_bir_lowering=False, debug=False,
                   num_devices=NCORES)
    qT = nc.dram_tensor("qT", [NB, D, S], F32R, kind="ExternalInput").ap()
    kT = nc.dram_tensor("kT", [NB, D, S], F32R, kind="ExternalInput").ap()
    vm = nc.dram_tensor("vm", [NB, S, W], F32R, kind="ExternalInput").ap()
    out = nc.dram_tensor("outT", [NB, D, S], F32, kind="ExternalOutput").ap()

    with tile.TileContext(nc) as tc:
        with (
            tc.tile_pool(name="singles", bufs=1) as singles,
            tc.tile_pool(name="qk", bufs=2) as qk_pool,
            tc.tile_pool(name="vmp", bufs=2) as vm_pool,
            tc.tile_pool(name="ex", bufs=4) as ex_pool,
            tc.tile_pool(name="so", bufs=3) as so_pool,
            tc.tile_pool(name="rcp", bufs=3) as rcp_pool,
            tc.tile_pool(name="scr", bufs=3, space="DRAM") as scr_pool,
            tc.tile_pool(name="rbc", bufs=3) as rbc_pool,
            tc.tile_pool(name="osb", bufs=3) as osb_pool,
            tc.tile_pool(name="ps_s", bufs=2, space="PSUM") as ps_pool,
            tc.tile_pool(name="ps_o", bufs=1, space="PSUM") as po_pool,
            tc.tile_pool(name="ps_t", bufs=2, space="PSUM") as pt_pool,
        ):
            # exp(s/8 - 3): the -3 bounds the denominator's fp16 range; it
            # cancels between numerator and denominator.
            bias_t = singles.tile([P, 1], F32)
            nc.vector.memset(bias_t, -3.0)
            # all-ones column; sliced per-partition so the transpose's two
            # operands share a base partition
            ident_t = singles.tile([W, 1], F32)
            nc.vector.memset(ident_t, 1.0)

            for s in range(NB):
                jt = jt_counts[s]
                qT_t = qk_pool.tile([D, S], F32R, tag="qT")
                kT_t = qk_pool.tile([D, S], F32R, tag="kT")
                nc.sync.dma_start(out=qT_t, in_=qT[s])
                nc.sync.dma_start(out=kT_t[:, 0:jt * P], in_=kT[s, :, 0:jt * P])
                # All key tiles of vm in one DMA: [128, jt*65], tile j at
                # columns [j*65, (j+1)*65).
                vm_t = vm_pool.tile([P, NJT * W], F32R, tag="vm", name="vm_t")
                nc.sync.dma_start(
                    out=vm_t.rearrange("p (j w) -> p j w", w=W)[:, 0:jt, :],
                    in_=vm[s, 0:jt * P, :].rearrange("(j p) w -> p j w", p=P),
                )
                # [v|mask]^T-weighted sums: rows 0..63 unnormalized outT,
                # row 64 the softmax denominator.  One accumulation group
                # per PSUM bank (cols 0:512 and 512:1024), spanning all j.
                po = po_pool.tile([W, S], F32, name="po")

                def emit_av(ex_j, j, jt=jt, po=po, vm_t=vm_t):
                    for half in range(2):
                        nc.tensor.matmul(
                            po[:, half * 512:(half + 1) * 512],
                            lhsT=vm_t[:, j * W:(j + 1) * W],
                            rhs=ex_j[:, half * 512:(half + 1) * 512],
                            start=(j == 0), stop=(j == jt - 1),
                        )

                # Scores/exp run one j ahead of the attn@v accumulation so
                # the PE never sits behind ScalarE in its own queue.
                prev = None
                for j in range(jt):
                    ps = ps_pool.tile([P, S], F32, tag="ps")
                    for half in range(2):
                        nc.tensor.matmul(
                            ps[:, half * 512:(half + 1) * 512],
                            lhsT=kT_t[:, j * P:(j + 1) * P],
                            rhs=qT_t[:, half * 512:(half + 1) * 512],
                            start=True, stop=True,
                        )
                    ex = ex_pool.tile([P, S], F32R, tag="ex", name="ex")
                    nc.scalar.activation(out=ex, in_=ps,
                                         func=mybir.ActivationFunctionType.Exp,
                                         scale=0.125, bias=bias_t)
                    if prev is not None:
                        emit_av(*prev)
                    prev = (ex, j)
                emit_av(*prev)

                # 1/denominator (18-bit approx is ~5x faster than the
                # iterative divide and den is a tame [1.1, 127] range), then
                # replicate across the 64 output partitions on GpSimd.
                # Free po as fast as possible: one DVE copy moves the whole
                # accumulator to SBUF, so the next batch's matmuls only wait
                # ~1.2us instead of on the full normalization chain.
                ub = so_pool.tile([W, S], F32, tag="ub", name="ub")
                nc.vector.tensor_copy(out=ub, in_=po)
                # DVE reciprocal runs 8 cycles/element serially along the
                # free dim ([1, 1024] would be ~6.5us), so fold the
                # denominator row onto 128 partitions first: copy it to fp16
                # (cheap PE weight loads), transpose 128-wide slices into one
                # PSUM bank, invert as [128, 8] (~0.2us), then bounce through
                # DRAM to restore row layout broadcast over 64 partitions
                # (0-stride reads are legal on DRAM sources).
                pt = pt_pool.tile([P, S // P], F32, tag="pt", name="pt")
                for c in range(S // P):
                    nc.tensor.transpose(pt[:, c:c + 1],
                                        ub[D:W, c * P:(c + 1) * P],
                                        ident_t[D:W, :])
                r128 = rcp_pool.tile([P, S // P], F32, tag="r128", name="r128")
                nc.vector.reciprocal(out=r128, in_=pt)
                scr2 = scr_pool.tile([1, S], F32, tag="scr2", name="scr2")
                scr2_pc = bass.AP(tensor=scr2.tensor, offset=scr2.offset,
                                  ap=[[1, P], [P, S // P]])
                nc.gpsimd.dma_start(out=scr2_pc, in_=r128)
                rbc = rbc_pool.tile([D, S], F32, tag="rbc", name="rbc")
                bcast_src = bass.AP(tensor=scr2.tensor, offset=scr2.offset,
                                    ap=[[0, D], [1, S]])
                nc.gpsimd.dma_start(out=rbc, in_=bcast_src)
                osb = osb_pool.tile([D, S], F32, tag="osb", name="osb")
                nc.vector.tensor_mul(osb, ub[0:D, :], rbc)
                nc.sync.dma_start(out=out[s], in_=osb)
    nc.compile()
    return nc


def kernel(q, k, v, valid_lens):
    global LAST_RESULTS
    q = np.array(q, dtype=np.float32, copy=True)
    k = np.asarray(k, dtype=np.float32)
    v = np.asarray(v, dtype=np.float32)
    vl = np.asarray(valid_lens).astype(np.int64)

    # valid_len == 0: reference's softmax over an all-masked row is uniform.
    # Zeroed q gives scores == 0 -> exp == 1 over all (unmasked) keys: same.
    valid_eff = np.where(vl <= 0, S, np.minimum(vl, S))
    q[vl <= 0] = 0.0

    mask = (np.arange(S)[None, :] < valid_eff[:, None]).astype(np.float32)
    qT = np.ascontiguousarray(q.transpose(0, 2, 1))
    kT = np.ascontiguousarray(k.transpose(0, 2, 1))
    vm = np.concatenate([v * mask[:, :, None], mask[:, :, None]], axis=2)
    vm = np.ascontiguousarray(vm, dtype=np.float32)

    # Rank-sort batches by effective length; slot s takes ranks [8s, 8s+8),
    # one per core, so the baked per-slot tile count wastes little work.
    order = np.argsort(-valid_eff, kind="stable")
    assign = order.reshape(NB, NCORES)[::-1]  # [slot, core] -> batch index
    # Ascending tile counts: small batches run first so their epilogue
    # chains hide under later compute; only the last epilogue is exposed.
    jt_counts = tuple(
        int(np.ceil(valid_eff[assign[s]].max() / P)) for s in range(NB)
    )

    nc = _program_cache.get(jt_counts)
    if nc is None:
        nc = _build_program(jt_counts)
        _program_cache[jt_counts] = nc

    in_maps = []
    for c in range(NCORES):
        bs = assign[:, c]
        in_maps.append({
            "qT": np.ascontiguousarray(qT[bs]),
            "kT": np.ascontiguousarray(kT[bs]),
            "vm": np.ascontiguousarray(vm[bs]),
        })
    res = bass_utils.run_bass_kernel_spmd(
        nc, in_maps, core_ids=list(range(NCORES)), trace=TRACE,
    )
    LAST_RESULTS = res

    out = np.empty((B, S, D), dtype=np.float32)
    for c in range(NCORES):
        o = res.results[c]["outT"]  # [NB, D, S]
        for s in range(NB):
            out[assign[s, c]] = o[s].T
    return out


# revision 31
# speedup vs baseline: 1.1677x; 1.0085x over previous
"""Masked dot-product attention on 8 Trainium2 NeuronCores.

Problem: q,k,v [64, 1024, 64] f32, valid_lens [64] int32.
  scores = q @ k^T / 8, mask keys >= valid_len to -1e6, softmax, @ v.

Strategy (per core: 8 batches, pure data parallelism, no collectives):
  - Host prep: pre-transpose q,k to [D, S] (contraction dim on partitions),
    pre-zero v rows past valid_len and append the 0/1 mask as a 65th column
    (vm).  The masked softmax denominator then falls out of the same matmul
    that computes attn @ v.  valid_len==0 batches reproduce the reference's
    uniform-softmax by zeroing q (scores==0) and unmasking all keys.
  - Matmul dtypes, chosen off measured TRN2 PE rates: all matmuls run
    f32r x f32r (fp32 bit layout at 1.5 cycles/row vs fp32's 4; bf16 would
    be 1 cycle/row but its 8-bit mantissa costs ~2e-3 output error), so
    q/k/v/exp are never quantized below fp32.
  - Device, per key-tile j: scoresT[j,q] = kT_tile.T.T @ qT (keys on
    partitions, fp32 PSUM), exp on ScalarE (scale=1/8, bias=-3 bounds the
    fp16 range; numerator and denominator share it so it cancels), then
    po[65, Sq] += vm_tile.T.T @ expT accumulated over j in fp32 PSUM.
    No max-subtraction: scores are O(1) (q,k ~ N(0,1), d=64), and masked
    columns are excluded by the mask column/zeroed v rows, not by -1e6.
  - Transpose-free epilogue: reciprocal_approx_fast over the denominator
    row (PSUM -> SBUF), GpSimd partition_broadcast across 64 partitions,
    one tensor_tensor multiply normalizes the [64, Sq] block, DMA out in
    [d, q] layout; the host transposes each batch during the gather.
  - DMA dispatch is the hidden serial resource (~0.6us per dma_start on a
    sequencer): inputs ride the Sync queue, epilogue DMAs the GpSimd queue.
  - Per-batch key tiles are truncated to ceil(valid/128): masked tail tiles
    contribute exactly zero, so they are skipped.  Batches are rank-sorted by
    valid_len and dealt one per core per slot so every core runs the same
    baked schedule with minimal padding.
"""

import numpy as np

import concourse.bass as bass
import concourse.bacc as bacc
import concourse.tile as tile
from concourse import mybir
from concourse import bass_utils

B, S, D = 64, 1024, 64
NCORES = 8
NB = B // NCORES  # batch slots per core
P = 128
NJT = S // P  # max key tiles per batch
W = D + 1  # v columns + mask column
F32 = mybir.dt.float32
F32R = mybir.dt.float32r
F16 = mybir.dt.float16

TRACE = False  # set by test harness to capture an NTFF profile
LAST_RESULTS = None  # BassKernelResults stash for the harness

_program_cache = {}


def _build_program(jt_counts):
    nc = bacc.Bacc("TRN2", target_bir_lowering=False, debug=False,
                   num_devices=NCORES)
    qT = nc.dram_tensor("qT", [NB, D, S], F32R, kind="ExternalInput").ap()
    kT = nc.dram_tensor("kT", [NB, D, S], F32R, kind="ExternalInput").ap()
    vm = nc.dram_tensor("vm", [NB, S, W], F32R, kind="ExternalInput").ap()
    out = nc.dram_tensor("outT", [NB, D, S], F32, kind="ExternalOutput").ap()

    with tile.TileContext(nc) as tc:
        with (
            tc.tile_pool(name="singles", bufs=1) as singles,
            tc.tile_pool(name="qk", bufs=2) as qk_pool,
            tc.tile_pool(name="vmp", bufs=2) as vm_pool,
            tc.tile_pool(name="ex", bufs=4) as ex_pool,
            tc.tile_pool(name="so", bufs=3) as so_pool,
            tc.tile_pool(name="rcp", bufs=3) as rcp_pool,
            tc.tile_pool(name="scr", bufs=3, space="DRAM") as scr_pool,
            tc.tile_pool(name="rbc", bufs=3) as rbc_pool,
            tc.tile_pool(name="osb", bufs=3) as osb_pool,
            tc.tile_pool(name="ps_s", bufs=2, space="PSUM") as ps_pool,
            tc.tile_pool(name="ps_o", bufs=1, space="PSUM") as po_pool,
            tc.tile_pool(name="ps_t", bufs=2, space="PSUM") as pt_pool,
        ):
            # exp(s/8 - 3): the -3 bounds the denominator's fp16 range; it
            # cancels between numerator and denominator.
            bias_t = singles.tile([P, 1], F32)
            nc.vector.memset(bias_t, -3.0)
            # all-ones column; sliced per-partition so the transpose's two
            # operands share a base partition
            ident_t = singles.tile([W, 1], F32)
            nc.vector.memset(ident_t, 1.0)

            pending = None
            for s in range(NB):
                jt = jt_counts[s]
                qT_t = qk_pool.tile([D, S], F32R, tag="qT")
                kT_t = qk_pool.tile([D, S], F32R, tag="kT")
                nc.sync.dma_start(out=qT_t, in_=qT[s])
                nc.sync.dma_start(out=kT_t[:, 0:jt * P], in_=kT[s, :, 0:jt * P])
                # All key tiles of vm in one DMA: [128, jt*65], tile j at
                # columns [j*65, (j+1)*65).
                vm_t = vm_pool.tile([P, NJT * W], F32R, tag="vm", name="vm_t")
                nc.sync.dma_start(
                    out=vm_t.rearrange("p (j w) -> p j w", w=W)[:, 0:jt, :],
                    in_=vm[s, 0:jt * P, :].rearrange("(j p) w -> p j w", p=P),
                )
                # [v|mask]^T-weighted sums: rows 0..63 unnormalized outT,
                # row 64 the softmax denominator.  One accumulation group
                # per PSUM bank (cols 0:512 and 512:1024), spanning all j.
                po = po_pool.tile([W, S], F32, name="po")

                def emit_av(ex_j, j, jt=jt, po=po, vm_t=vm_t):
                    for half in range(2):
                        nc.tensor.matmul(
                            po[:, half * 512:(half + 1) * 512],
                            lhsT=vm_t[:, j * W:(j + 1) * W],
                            rhs=ex_j[:, half * 512:(half + 1) * 512],
                            start=(j == 0), stop=(j == jt - 1),
                        )

                # Scores/exp run one j ahead of the attn@v accumulation so
                # the PE never sits behind ScalarE in its own queue.
                prev = None
                for j in range(jt):
                    ps = ps_pool.tile([P, S], F32, tag="ps")
                    for half in range(2):
                        nc.tensor.matmul(
                            ps[:, half * 512:(half + 1) * 512],
                            lhsT=kT_t[:, j * P:(j + 1) * P],
                            rhs=qT_t[:, half * 512:(half + 1) * 512],
                            start=True, stop=True,
                        )
                    ex = ex_pool.tile([P, S], F32R, tag="ex", name="ex")
                    nc.scalar.activation(out=ex, in_=ps,
                                         func=mybir.ActivationFunctionType.Exp,
                                         scale=0.125, bias=bias_t)
                    if prev is not None:
                        emit_av(*prev)
                    prev = (ex, j)
                emit_av(*prev)

                # 1/denominator (18-bit approx is ~5x faster than the
                # iterative divide and den is a tame [1.1, 127] range), then
                # replicate across the 64 output partitions on GpSimd.
                # Free po as fast as possible: one DVE copy moves the whole
                # accumulator to SBUF, so the next batch's matmuls only wait
                # ~1.2us instead of on the full normalization chain.
                ub = so_pool.tile([W, S], F32, tag="ub", name="ub")
                nc.vector.tensor_copy(out=ub, in_=po)
                # Previous batch's normalize+store: deferred a full batch so
                # its broadcast DMA chain has certainly landed -- the in-order
                # DVE queue then never blocks this batch's po-freeing copy.
                if pending is not None:
                    p_ub, p_rbc, p_s = pending
                    osb = osb_pool.tile([D, S], F32, tag="osb", name="osb")
                    nc.vector.tensor_mul(osb, p_ub[0:D, :], p_rbc)
                    nc.sync.dma_start(out=out[p_s], in_=osb)
                # DVE reciprocal runs 8 cycles/element serially along the
                # free dim ([1, 1024] would be ~6.5us), so fold the
                # denominator row onto 128 partitions first: PE-transpose
                # 128-wide slices of the denominator row into one PSUM bank,
                # invert as [128, 8] (~0.2us), then bounce through DRAM to
                # restore row layout broadcast over 64 partitions (0-stride
                # reads are legal on DRAM sources).
                pt = pt_pool.tile([P, S // P], F32, tag="pt", name="pt")
                for c in range(S // P):
                    nc.tensor.transpose(pt[:, c:c + 1],
                                        ub[D:W, c * P:(c + 1) * P],
                                        ident_t[D:W, :])
                r128 = rcp_pool.tile([P, S // P], F32, tag="r128", name="r128")
                nc.vector.reciprocal(out=r128, in_=pt)
                scr2 = scr_pool.tile([1, S], F32, tag="scr2", name="scr2")
                scr2_pc = bass.AP(tensor=scr2.tensor, offset=scr2.offset,
                                  ap=[[1, P], [P, S // P]])
                nc.gpsimd.dma_start(out=scr2_pc, in_=r128)
                rbc = rbc_pool.tile([D, S], F32, tag="rbc", name="rbc")
                bcast_src = bass.AP(tensor=scr2.tensor, offset=scr2.offset,
                                    ap=[[0, D], [1, S]])
                nc.gpsimd.dma_start(out=rbc, in_=bcast_src)
                pending = (ub, rbc, s)
            p_ub, p_rbc, p_s = pending
            osb = osb_pool.tile([D, S], F32, tag="osb", name="osb")
            nc.vector.tensor_mul(osb, p_ub[0:D, :], p_rbc)
            nc.sync.dma_start(out=out[p_s], in_=osb)
    nc.compile()
    return nc


def kernel(q, k, v, valid_lens):
    global LAST_RESULTS
    q = np.array(q, dtype=np.float32, copy=True)
    k = np.asarray(k, dtype=np.float32)
    v = np.asarray(v, dtype=np.float32)
    vl = np.asarray(valid_lens).astype(np.int64)

    # valid_len == 0: reference's softmax over an all-masked row is uniform.
    # Zeroed q gives scores == 0 -> exp == 1 over all (unmasked) keys: same.
    valid_eff = np.where(vl <= 0, S, np.minimum(vl, S))
    q[vl <= 0] = 0.0

    mask = (np.arange(S)[None, :] < valid_eff[:, None]).astype(np.float32)
    qT = np.ascontiguousarray(q.transpose(0, 2, 1))
    kT = np.ascontiguousarray(k.transpose(0, 2, 1))
    vm = np.concatenate([v * mask[:, :, None], mask[:, :, None]], axis=2)
    vm = np.ascontiguousarray(vm, dtype=np.float32)

    # Rank-sort batches by effective length; slot s takes ranks [8s, 8s+8),
    # one per core, so the baked per-slot tile count wastes little work.
    order = np.argsort(-valid_eff, kind="stable")
    assign = order.reshape(NB, NCORES)[::-1]  # [slot, core] -> batch index
    # Ascending tile counts: small batches run first so their epilogue
    # chains hide under later compute; only the last epilogue is exposed.
    jt_counts = tuple(
        int(np.ceil(valid_eff[assign[s]].max() / P)) for s in range(NB)
    )

    nc = _program_cache.get(jt_counts)
    if nc is None:
        nc = _build_program(jt_counts)
        _program_cache[jt_counts] = nc

    in_maps = []
    for c in range(NCORES):
        bs = assign[:, c]
        in_maps.append({
            "qT": np.ascontiguousarray(qT[bs]),
            "kT": np.ascontiguousarray(kT[bs]),
            "vm": np.ascontiguousarray(vm[bs]),
        })
    res = bass_utils.run_bass_kernel_spmd(
        nc, in_maps, core_ids=list(range(NCORES)), trace=TRACE,
    )
    LAST_RESULTS = res

    out = np.empty((B, S, D), dtype=np.float32)
    for c in range(NCORES):
        o = res.results[c]["outT"]  # [NB, D, S]
        for s in range(NB):
            out[assign[s, c]] = o[s].T
    return out


# revision 32
# speedup vs baseline: 1.9997x; 1.7125x over previous
"""Masked dot-product attention on 8 Trainium2 NeuronCores.

Problem: q,k,v [64, 1024, 64] f32, valid_lens [64] int32.
  scores = q @ k^T / 8, mask keys >= valid_len to -1e6, softmax, @ v.

Strategy (per core: 8 batches, pure data parallelism, no collectives):
  - Host prep: pre-transpose q,k to [D, S] (contraction dim on partitions),
    pre-zero v rows past valid_len and append the 0/1 mask as a 65th column
    (vm).  The masked softmax denominator then falls out of the same matmul
    that computes attn @ v.  valid_len==0 batches reproduce the reference's
    uniform-softmax by zeroing q (scores==0) and unmasking all keys.
  - Device, per key-tile j: scoresT[j,q] = kT_tile.T.T @ qT with f32r
    operands (keys on partitions, fp32 PSUM; f32r streams ~1.5 cycles/row
    vs fp32's 4 at near-fp32 accuracy), exp on ScalarE (scale=1/8, bias=-3
    bounds the fp16 range; it cancels between numerator and denominator),
    output written fp16 (~3e-4; bf16 would cost ~2e-3).
  - attn@v runs with the exp'd tile as the stationary operand:
    po[128q, 65] += expT_chunk.T.T @ [v|mask]_tile per 128-query chunk,
    fp32 PSUM.  The fp16 weights ride the fast weight-load path and only 65
    columns stream per chunk, and the result lands queries-on-partitions:
    the softmax division is then a cheap [128, 4] reciprocal plus
    per-partition tensor_scalar multiplies -- no transposes, no broadcasts.
  - Chunk accumulation groups sharing a PSUM bank run sequentially (a
    group's start clears has_written for the whole bank), so the qc loop is
    outer and all exp tiles of a batch stay resident in SBUF.
  - DMA dispatch is the hidden serial resource (~0.6us per dma_start per
    sequencer): one vm load and one output store per batch, inputs on the
    Sync queue, outputs on the GpSimd queue.
  - Per-batch key tiles are truncated to ceil(valid/128): masked tail tiles
    contribute exactly zero, so they are skipped.  Batches are rank-sorted
    by valid_len and dealt one per core per slot (same baked schedule on
    every core), shortest slots first so epilogues hide under later compute.
"""

import numpy as np

import concourse.bacc as bacc
import concourse.tile as tile
from concourse import mybir
from concourse import bass_utils

B, S, D = 64, 1024, 64
NCORES = 8
NB = B // NCORES  # batch slots per core
P = 128
NJT = S // P  # max key tiles per batch
W = D + 1  # v columns + mask column
F32 = mybir.dt.float32
F32R = mybir.dt.float32r
F16 = mybir.dt.float16

TRACE = False  # set by test harness to capture an NTFF profile
LAST_RESULTS = None  # BassKernelResults stash for the harness

_program_cache = {}


def _build_program(jt_counts):
    nc = bacc.Bacc("TRN2", target_bir_lowering=False, debug=False,
                   num_devices=NCORES)
    qT = nc.dram_tensor("qT", [NB, D, S], F32R, kind="ExternalInput").ap()
    kT = nc.dram_tensor("kT", [NB, D, S], F32R, kind="ExternalInput").ap()
    vm = nc.dram_tensor("vm", [NB, S, W], F16, kind="ExternalInput").ap()
    out = nc.dram_tensor("out", [NB, S, D], F32, kind="ExternalOutput").ap()

    with tile.TileContext(nc) as tc:
        with (
            tc.tile_pool(name="singles", bufs=1) as singles,
            tc.tile_pool(name="qk", bufs=2) as qk_pool,
            tc.tile_pool(name="vmp", bufs=2) as vm_pool,
            tc.tile_pool(name="ex", bufs=NJT + 4) as ex_pool,
            tc.tile_pool(name="osb", bufs=2) as osb_pool,
            tc.tile_pool(name="rec", bufs=4) as rec_pool,
            tc.tile_pool(name="ps_s", bufs=2, space="PSUM") as ps_pool,
            tc.tile_pool(name="ps_o", bufs=2, space="PSUM") as po_pool,
        ):
            # exp(s/8 - 3): the -3 bounds the fp16 exp range; it cancels
            # between numerator and denominator.
            bias_t = singles.tile([P, 1], F32)
            nc.vector.memset(bias_t, -3.0)

            for s in range(NB):
                jt = jt_counts[s]
                qT_t = qk_pool.tile([D, S], F32R, tag="qT")
                kT_t = qk_pool.tile([D, S], F32R, tag="kT")
                nc.sync.dma_start(out=qT_t, in_=qT[s])
                nc.sync.dma_start(out=kT_t[:, 0:jt * P], in_=kT[s, :, 0:jt * P])
                # All key tiles of vm in one DMA: [128, jt*65], tile j at
                # columns [j*65, (j+1)*65).
                vm_t = vm_pool.tile([P, NJT * W], F16, tag="vm", name="vm_t")
                nc.sync.dma_start(
                    out=vm_t.rearrange("p (j w) -> p j w", w=W)[:, 0:jt, :],
                    in_=vm[s, 0:jt * P, :].rearrange("(j p) w -> p j w", p=P),
                )
                # Output accumulators: 8 query-chunks of [128, 65] (cols
                # 0..63 = unnormalized out rows, col 64 = denominator); a
                # 65-wide chunk can't cross a PSUM bank so they're split 4+4
                # over two banks.
                po = [po_pool.tile([P, 4 * W], F32, tag=f"po{h}",
                                   name=f"po{h}")
                      for h in range(2)]

                exs = []
                for j in range(jt):
                    ps = ps_pool.tile([P, S], F32, tag="ps")
                    for half in range(2):
                        nc.tensor.matmul(
                            ps[:, half * 512:(half + 1) * 512],
                            lhsT=kT_t[:, j * P:(j + 1) * P],
                            rhs=qT_t[:, half * 512:(half + 1) * 512],
                            start=True, stop=True,
                        )
                    ex = ex_pool.tile([P, S], F16, tag="ex", name="ex")
                    nc.scalar.activation(out=ex, in_=ps,
                                         func=mybir.ActivationFunctionType.Exp,
                                         scale=0.125, bias=bias_t)
                    exs.append(ex)
                # One pending accumulation group per PSUM bank at a time:
                # a group's start clears has_written for the whole bank, so
                # the 4 chunk groups sharing a bank run sequentially.
                for qc in range(8):
                    dst = po[qc // 4]
                    col = (qc % 4) * W
                    for j in range(jt):
                        nc.tensor.matmul(
                            dst[:, col:col + W],
                            lhsT=exs[j][:, qc * P:(qc + 1) * P],
                            rhs=vm_t[:, j * W:(j + 1) * W],
                            start=(j == 0), stop=(j == jt - 1),
                        )
                osb = osb_pool.tile([P, 8 * D], F32, tag="osb", name="osb")
                for h in range(2):
                    po3 = po[h].rearrange("p (c w) -> p c w", w=W)
                    recp = rec_pool.tile([P, 4], F32, tag="rec", name="recp")
                    nc.vector.reciprocal(out=recp, in_=po3[:, :, D])
                    for i in range(4):
                        qc = 4 * h + i
                        nc.vector.tensor_scalar_mul(
                            osb[:, qc * D:(qc + 1) * D],
                            po3[:, i, 0:D],
                            recp[:, i:i + 1],
                        )
                nc.gpsimd.dma_start(
                    out=out[s].rearrange("(c p) d -> p c d", p=P),
                    in_=osb.rearrange("p (c d) -> p c d", d=D),
                )
    nc.compile()
    return nc


def kernel(q, k, v, valid_lens):
    global LAST_RESULTS
    q = np.array(q, dtype=np.float32, copy=True)
    k = np.asarray(k, dtype=np.float32)
    v = np.asarray(v, dtype=np.float32)
    vl = np.asarray(valid_lens).astype(np.int64)

    # valid_len == 0: reference's softmax over an all-masked row is uniform.
    # Zeroed q gives scores == 0 -> exp == 1 over all (unmasked) keys: same.
    valid_eff = np.where(vl <= 0, S, np.minimum(vl, S))
    q[vl <= 0] = 0.0

    mask = (np.arange(S)[None, :] < valid_eff[:, None]).astype(np.float32)
    qT = np.ascontiguousarray(q.transpose(0, 2, 1))
    kT = np.ascontiguousarray(k.transpose(0, 2, 1))
    vm = np.concatenate([v * mask[:, :, None], mask[:, :, None]], axis=2)
    vm = np.ascontiguousarray(vm).astype(np.float16)

    # Rank-sort batches by effective length; slot s takes one batch of rank
    # group [8s, 8s+8) per core, so the baked per-slot tile count wastes
    # little work.  Shortest slots run first (see module docstring).
    order = np.argsort(-valid_eff, kind="stable")
    assign = order.reshape(NB, NCORES)[::-1]  # [slot, core] -> batch index
    jt_counts = tuple(
        int(np.ceil(valid_eff[assign[s]].max() / P)) for s in range(NB)
    )

    nc = _program_cache.get(jt_counts)
    if nc is None:
        nc = _build_program(jt_counts)
        _program_cache[jt_counts] = nc

    in_maps = []
    for c in range(NCORES):
        bs = assign[:, c]
        in_maps.append({
            "qT": np.ascontiguousarray(qT[bs]),
            "kT": np.ascontiguousarray(kT[bs]),
            "vm": np.ascontiguousarray(vm[bs]),
        })
    res = bass_utils.run_bass_kernel_spmd(
        nc, in_maps, core_ids=list(range(NCORES)), trace=TRACE,
    )
    LAST_RESULTS = res

    out = np.empty((B, S, D), dtype=np.float32)
    for c in range(NCORES):
        o = res.results[c]["out"]
        for s in range(NB):
            out[assign[s, c]] = o[s]
    return out
